# revision 1
# baseline (speedup 1.0000x reference)
"""Trainium2 Bass kernel for nn_ExpertModule (moe_routing).

Computation (per the reference):
  3 input banks (share_x, task_x0, task_x1), each [B=16384, H=512] f32.
  Each bank runs E=5 experts: o_e = relu(x @ W1_e + b1_e) @ W2_e + b2_e
  with W1_e [512,512], W2_e [512,128]. Output per bank: [E,B,OUT] viewed
  as [B, E, 1, OUT].

Strategy: data-parallel over B across 8 NeuronCores (2048 rows/core),
expert weights replicated. On-chip layout keeps the contraction dim on
SBUF partitions:
  - x is fed pre-transposed (xT: [H on partitions, B free], bf16)
  - GEMM1 computes hT = W1^T-free-dim matmuls -> psum [h' chunk, b]
  - ACT applies relu + per-partition bias b1, casts to bf16
  - GEMM2 computes oT [OUT on partitions, b] with W2 as the stationary
    operand, DVE adds per-partition bias b2 in the psum->sbuf copy.
Host transposes oT -> o and reshapes to the reference layout.
bf16 inputs keep the PE at 1 cycle/row (fp32 would be 4x slower); fp32
accumulation in PSUM bounds the error at ~3e-3 scale-relative absmax.
"""

import numpy as np
import ml_dtypes

B = 16384
H = 512
E = 5
T = 2
OUT = 128
NB = 3                 # input banks: share, task0, task1
NE = NB * E            # 15 expert instances
NCORES = 8
BSH = B // NCORES      # 2048 rows per core
P = 128
KC = H // P            # 4 contraction chunks
MC = H // P            # 4 h' chunks
NT = BSH // 512        # 4 b tiles of 512

BF16 = ml_dtypes.bfloat16

_compiled = None       # cached (nc, ) across calls


def _build_program():
    import concourse.mybir as mybir
    from concourse import bacc
    from concourse.tile import TileContext

    f32 = mybir.dt.float32
    bf16 = mybir.dt.bfloat16

    nc = bacc.Bacc("TRN2", target_bir_lowering=False, debug=False,
                   num_devices=NCORES)

    xt_d = nc.declare_dram_parameter("xt", [NB, P, KC, BSH], bf16, isOutput=False)
    w1_d = nc.declare_dram_parameter("w1", [NE, P, KC, H], bf16, isOutput=False)
    w2_d = nc.declare_dram_parameter("w2", [NE, P, KC, OUT], bf16, isOutput=False)
    b1_d = nc.declare_dram_parameter("b1", [P, NE, MC], f32, isOutput=False)
    b2_d = nc.declare_dram_parameter("b2", [P, NE], f32, isOutput=False)
    out_d = nc.declare_dram_parameter("out", [NE, P, BSH], f32, isOutput=True)

    with TileContext(nc) as tc:
        with (
            tc.tile_pool(name="xpool", bufs=1) as xpool,
            tc.tile_pool(name="consts", bufs=1) as consts,
            tc.tile_pool(name="w1pool", bufs=3) as w1pool,
            tc.tile_pool(name="w2pool", bufs=3) as w2pool,
            tc.tile_pool(name="hpool", bufs=2) as hpool,
            tc.tile_pool(name="opool", bufs=2) as opool,
            tc.tile_pool(name="ps1", bufs=5, space="PSUM") as ps1,
            tc.tile_pool(name="ps2", bufs=3, space="PSUM") as ps2,
        ):
            # Resident activations: all 3 banks of xT, split DMAs so the
            # first matmul group can start after ~512KB.
            xt_sb = xpool.tile([P, NB, KC, BSH], bf16)
            for bank in range(NB):
                for n in range(NT):
                    nc.sync.dma_start(
                        out=xt_sb[:, bank, :, n * 512:(n + 1) * 512],
                        in_=xt_d[bank][:, :, n * 512:(n + 1) * 512],
                    )
            b1_sb = consts.tile([P, NE, MC], f32)
            nc.sync.dma_start(out=b1_sb[:], in_=b1_d[:])
            b2_sb = consts.tile([P, NE], f32)
            nc.sync.dma_start(out=b2_sb[:], in_=b2_d[:])

            live = {}  # step i -> (h tile, w2 tile)
            for step in range(NE + 1):
                if step < NE:
                    i = step
                    bank = i // E
                    w1_sb = w1pool.tile([P, KC, H], bf16)
                    nc.sync.dma_start(out=w1_sb[:], in_=w1_d[i])
                    w2_sb = w2pool.tile([P, KC, OUT], bf16)
                    nc.sync.dma_start(out=w2_sb[:], in_=w2_d[i])
                    h_sb = hpool.tile([P, MC, BSH], bf16)
                    live[i] = (h_sb, w2_sb)
                    for m in range(MC):
                        for n in range(NT):
                            ps = ps1.tile([P, 512], f32)
                            for kc in range(KC):
                                nc.tensor.matmul(
                                    ps[:],
                                    w1_sb[:, kc, m * P:(m + 1) * P],
                                    xt_sb[:, bank, kc, n * 512:(n + 1) * 512],
                                    start=(kc == 0),
                                    stop=(kc == KC - 1),
                                )
                            nc.scalar.activation(
                                h_sb[:, m, n * 512:(n + 1) * 512],
                                ps[:],
                                mybir.ActivationFunctionType.Relu,
                                bias=b1_sb[:, i, m, None],
                            )
                if step > 0:
                    j = step - 1
                    h_sb, w2_sb = live.pop(j)
                    o_sb = opool.tile([P, BSH], f32)
                    for n in range(NT):
                        ps = ps2.tile([P, 512], f32)
                        for kc in range(KC):
                            nc.tensor.matmul(
                                ps[:],
                                w2_sb[:, kc, :],
                                h_sb[:, kc, n * 512:(n + 1) * 512],
                                start=(kc == 0),
                                stop=(kc == KC - 1),
                            )
                        nc.vector.tensor_add(
                            o_sb[:, n * 512:(n + 1) * 512],
                            ps[:],
                            b2_sb[:, j, None].to_broadcast([P, 512]),
                        )
                    nc.sync.dma_start(out=out_d[j], in_=o_sb[:])

    nc.compile()
    return nc


def _prep_inputs(share_x, task_x0, task_x1, share_W1, share_b1, share_W2,
                 share_b2, task_W1, task_b1, task_W2, task_b2):
    X = np.stack([np.asarray(share_x), np.asarray(task_x0),
                  np.asarray(task_x1)]).astype(np.float32)      # [3, B, H]
    Xb = X.astype(BF16)
    Xt = np.ascontiguousarray(Xb.transpose(0, 2, 1))            # [3, H, B]
    Xt = Xt.reshape(NB, KC, P, B).transpose(0, 2, 1, 3)         # [3, P, KC, B]

    W1 = np.concatenate([np.asarray(share_W1),
                         np.asarray(task_W1).reshape(T * E, H, H)])  # [15,H,H]
    w1h = np.ascontiguousarray(
        W1.astype(BF16).reshape(NE, KC, P, H).transpose(0, 2, 1, 3))
    W2 = np.concatenate([np.asarray(share_W2),
                         np.asarray(task_W2).reshape(T * E, H, OUT)])
    w2h = np.ascontiguousarray(
        W2.astype(BF16).reshape(NE, KC, P, OUT).transpose(0, 2, 1, 3))

    B1 = np.concatenate([np.asarray(share_b1),
                         np.asarray(task_b1).reshape(T * E, H)]).astype(np.float32)
    b1h = np.ascontiguousarray(B1.reshape(NE, MC, P).transpose(2, 0, 1))
    B2 = np.concatenate([np.asarray(share_b2),
                         np.asarray(task_b2).reshape(T * E, OUT)]).astype(np.float32)
    b2h = np.ascontiguousarray(B2.T)

    in_maps = []
    for c in range(NCORES):
        xt_c = np.ascontiguousarray(Xt[:, :, :, c * BSH:(c + 1) * BSH])
        in_maps.append({"xt": xt_c, "w1": w1h, "w2": w2h, "b1": b1h, "b2": b2h})
    return in_maps


def _assemble(results):
    outs = np.stack([results[c]["out"] for c in range(NCORES)])  # [8,15,P,BSH]
    # outs[c, i, p, b] = o_i[c*BSH + b, p] -> A[i, B, OUT]
    A = np.ascontiguousarray(outs.transpose(1, 0, 3, 2)).reshape(NE, B, OUT)
    banks = []
    for bank in range(NB):
        o = A[bank * E:(bank + 1) * E]                    # [E, B, OUT]
        banks.append(o.reshape(-1, E, 1, OUT))            # [B, E, 1, OUT]
    return tuple(banks)


def kernel(**inputs):
    global _compiled
    from concourse.bass_utils import run_bass_kernel_spmd

    if _compiled is None:
        _compiled = _build_program()
    nc = _compiled

    in_maps = _prep_inputs(**inputs)
    res = run_bass_kernel_spmd(nc, in_maps, list(range(NCORES)))
    return _assemble(res.results)


# revision 5
# speedup vs baseline: 13.6329x; 13.6329x over previous
"""Trainium2 Bass kernel for nn_ExpertModule (moe_routing).

Computation (per the reference):
  3 input banks (share_x, task_x0, task_x1), each [B=16384, H=512] f32.
  Each bank runs E=5 experts: o_e = relu(x @ W1_e + b1_e) @ W2_e + b2_e
  with W1_e [512,512], W2_e [512,128]. Output per bank: [E,B,OUT] viewed
  as [B, E, 1, OUT].

Strategy: data-parallel over B across 8 NeuronCores (2048 rows/core),
expert weights replicated. On-chip layout keeps the contraction dim on
SBUF partitions:
  - x is fed pre-transposed (xT: [H on partitions, B free], bf16)
  - GEMM1 computes hT = W1^T-free-dim matmuls -> psum [h' chunk, b]
  - ACT applies relu + per-partition bias b1, casts to bf16
  - GEMM2 computes oT [OUT on partitions, b] with W2 as the stationary
    operand, DVE adds per-partition bias b2 in the psum->sbuf copy.
Host transposes oT -> o and reshapes to the reference layout.
bf16 inputs keep the PE at 1 cycle/row (fp32 would be 4x slower); fp32
accumulation in PSUM bounds the error at ~3e-3 scale-relative absmax.
"""

import numpy as np
import ml_dtypes

B = 16384
H = 512
E = 5
T = 2
OUT = 128
NB = 3                 # input banks: share, task0, task1
NE = NB * E            # 15 expert instances
NCORES = 8
BSH = B // NCORES      # 2048 rows per core
P = 128
KC = H // P            # 4 contraction chunks
MC = H // P            # 4 h' chunks
NT = BSH // 512        # 4 b tiles of 512

BF16 = ml_dtypes.bfloat16

_compiled = None       # cached (nc, ) across calls


def _build_program(repeat=None):
    """Build the per-core program. repeat=None emits the plain kernel;
    repeat=R wraps the body in a hardware For_i loop (timing rig only).
    """
    import concourse.mybir as mybir
    from concourse import bacc
    from concourse.tile import TileContext
    from contextlib import nullcontext

    f32 = mybir.dt.float32
    bf16 = mybir.dt.bfloat16

    nc = bacc.Bacc("TRN2", target_bir_lowering=False, debug=False,
                   num_devices=NCORES)

    xt_d = nc.declare_dram_parameter("xt", [NB, P, KC, BSH], bf16, isOutput=False)
    w1_d = nc.declare_dram_parameter("w1", [NE, P, KC, H], bf16, isOutput=False)
    w2_d = nc.declare_dram_parameter("w2", [NE, P, KC, OUT], bf16, isOutput=False)
    b1_d = nc.declare_dram_parameter("b1", [P, NE, MC], f32, isOutput=False)
    b2_d = nc.declare_dram_parameter("b2", [P, NE], f32, isOutput=False)
    out_d = nc.declare_dram_parameter("out", [NE, P, BSH], f32, isOutput=True)

    with TileContext(nc) as tc:
        loop_ctx = (tc.For_i(0, repeat, 1, hint_engines=(mybir.EngineType.PE,))
                    if repeat is not None else nullcontext())
        with (
            loop_ctx,
            tc.tile_pool(name="xpool", bufs=1) as xpool,
            tc.tile_pool(name="consts", bufs=1) as consts,
            tc.tile_pool(name="w1pool", bufs=3) as w1pool,
            tc.tile_pool(name="w2pool", bufs=3) as w2pool,
            tc.tile_pool(name="hpool", bufs=2) as hpool,
            tc.tile_pool(name="opool", bufs=2) as opool,
            tc.tile_pool(name="ps1", bufs=5, space="PSUM") as ps1,
            tc.tile_pool(name="ps2", bufs=3, space="PSUM") as ps2,
        ):
            # xT lives in per-(bank, n-tile) tiles so the first matmul
            # group only waits on its own 512KB slice, not all 6MB.
            xt_sb = {}

            def load_x(bank, n):
                t = xpool.tile([P, KC, 512], bf16, tag=f"xt{bank}_{n}")
                nc.sync.dma_start(
                    out=t[:], in_=xt_d[bank][:, :, n * 512:(n + 1) * 512])
                xt_sb[(bank, n)] = t

            # Critical-path DMAs first: first expert's W1, then bank0 x.
            w1_first = w1pool.tile([P, KC, H], bf16, tag="w1")
            nc.sync.dma_start(out=w1_first[:], in_=w1_d[0])
            load_x(0, 0)
            b1_sb = consts.tile([P, NE, MC], f32)
            nc.sync.dma_start(out=b1_sb[:], in_=b1_d[:])
            b2_sb = consts.tile([P, NE], f32)
            nc.sync.dma_start(out=b2_sb[:], in_=b2_d[:])
            for n in range(1, NT):
                load_x(0, n)

            def gemm1_group(i, bank, m, n, w1_sb, h_sb):
                ps = ps1.tile([P, 512], f32)
                for kc in range(KC):
                    nc.tensor.matmul(
                        ps[:],
                        w1_sb[:, kc, m * P:(m + 1) * P],
                        xt_sb[(bank, n)][:, kc, :],
                        start=(kc == 0),
                        stop=(kc == KC - 1),
                    )
                nc.scalar.activation(
                    h_sb[:, m, n * 512:(n + 1) * 512],
                    ps[:],
                    mybir.ActivationFunctionType.Relu,
                    bias=b1_sb[:, i, m, None],
                )

            def gemm2_group(j, n, w2_sb, h_sb, o_sb):
                ps = ps2.tile([P, 512], f32)
                for kc in range(KC):
                    nc.tensor.matmul(
                        ps[:],
                        w2_sb[:, kc, :],
                        h_sb[:, kc, n * 512:(n + 1) * 512],
                        start=(kc == 0),
                        stop=(kc == KC - 1),
                    )
                nc.vector.tensor_add(
                    o_sb[:, n * 512:(n + 1) * 512],
                    ps[:],
                    b2_sb[:, j, None].to_broadcast([P, 512]),
                )
                nc.sync.dma_start(
                    out=out_d[j][:, n * 512:(n + 1) * 512],
                    in_=o_sb[:, n * 512:(n + 1) * 512],
                )

            live = {}  # step i -> (h tile, w2 tile)
            for step in range(NE + 1):
                if step < NE:
                    i = step
                    bank = i // E
                    if i == 0:
                        w1_sb = w1_first
                    else:
                        w1_sb = w1pool.tile([P, KC, H], bf16, tag="w1")
                        nc.sync.dma_start(out=w1_sb[:], in_=w1_d[i])
                    w2_sb = w2pool.tile([P, KC, OUT], bf16)
                    nc.sync.dma_start(out=w2_sb[:], in_=w2_d[i])
                    # Stream later banks' x while early experts compute.
                    if step == 1:
                        for n in range(NT):
                            load_x(1, n)
                    elif step == 5:
                        for n in range(NT):
                            load_x(2, n)
                    h_sb = hpool.tile([P, MC, BSH], bf16)
                    live[i] = (h_sb, w2_sb)
                    if i < NE - 1:
                        for m in range(MC):
                            for n in range(NT):
                                gemm1_group(i, bank, m, n, w1_sb, h_sb)
                    else:
                        # Last expert: n-outer so its GEMM2 groups can
                        # chase the relu wavefront and shorten the tail.
                        o_sb = opool.tile([P, BSH], f32)
                        live[i] = (h_sb, w2_sb, o_sb)
                        for n in range(NT):
                            for m in range(MC):
                                gemm1_group(i, bank, m, n, w1_sb, h_sb)
                            if n > 0:
                                gemm2_group(i, n - 1, w2_sb, h_sb, o_sb)
                if step > 0:
                    j = step - 1
                    if j < NE - 1:
                        h_sb, w2_sb = live.pop(j)
                        o_sb = opool.tile([P, BSH], f32)
                        for n in range(NT):
                            gemm2_group(j, n, w2_sb, h_sb, o_sb)
                    else:
                        h_sb, w2_sb, o_sb = live.pop(j)
                        gemm2_group(j, NT - 1, w2_sb, h_sb, o_sb)

    nc.compile()
    return nc


def _prep_inputs(share_x, task_x0, task_x1, share_W1, share_b1, share_W2,
                 share_b2, task_W1, task_b1, task_W2, task_b2):
    X = np.stack([np.asarray(share_x), np.asarray(task_x0),
                  np.asarray(task_x1)]).astype(np.float32)      # [3, B, H]
    Xb = X.astype(BF16)
    Xt = np.ascontiguousarray(Xb.transpose(0, 2, 1))            # [3, H, B]
    Xt = Xt.reshape(NB, KC, P, B).transpose(0, 2, 1, 3)         # [3, P, KC, B]

    W1 = np.concatenate([np.asarray(share_W1),
                         np.asarray(task_W1).reshape(T * E, H, H)])  # [15,H,H]
    w1h = np.ascontiguousarray(
        W1.astype(BF16).reshape(NE, KC, P, H).transpose(0, 2, 1, 3))
    W2 = np.concatenate([np.asarray(share_W2),
                         np.asarray(task_W2).reshape(T * E, H, OUT)])
    w2h = np.ascontiguousarray(
        W2.astype(BF16).reshape(NE, KC, P, OUT).transpose(0, 2, 1, 3))

    B1 = np.concatenate([np.asarray(share_b1),
                         np.asarray(task_b1).reshape(T * E, H)]).astype(np.float32)
    b1h = np.ascontiguousarray(B1.reshape(NE, MC, P).transpose(2, 0, 1))
    B2 = np.concatenate([np.asarray(share_b2),
                         np.asarray(task_b2).reshape(T * E, OUT)]).astype(np.float32)
    b2h = np.ascontiguousarray(B2.T)

    in_maps = []
    for c in range(NCORES):
        xt_c = np.ascontiguousarray(Xt[:, :, :, c * BSH:(c + 1) * BSH])
        in_maps.append({"xt": xt_c, "w1": w1h, "w2": w2h, "b1": b1h, "b2": b2h})
    return in_maps


def _assemble(results):
    outs = np.stack([results[c]["out"] for c in range(NCORES)])  # [8,15,P,BSH]
    # outs[c, i, p, b] = o_i[c*BSH + b, p] -> A[i, B, OUT]
    A = np.ascontiguousarray(outs.transpose(1, 0, 3, 2)).reshape(NE, B, OUT)
    banks = []
    for bank in range(NB):
        o = A[bank * E:(bank + 1) * E]                    # [E, B, OUT]
        banks.append(o.reshape(-1, E, 1, OUT))            # [B, E, 1, OUT]
    return tuple(banks)


def kernel(**inputs):
    global _compiled
    from concourse.bass_utils import run_bass_kernel_spmd

    if _compiled is None:
        _compiled = _build_program()
    nc = _compiled

    in_maps = _prep_inputs(**inputs)
    res = run_bass_kernel_spmd(nc, in_maps, list(range(NCORES)))
    return _assemble(res.results)


# revision 6
# speedup vs baseline: 36.8742x; 2.7048x over previous
"""Trainium2 Bass kernel for nn_ExpertModule (moe_routing).

Computation (per the reference):
  3 input banks (share_x, task_x0, task_x1), each [B=16384, H=512] f32.
  Each bank runs E=5 experts: o_e = relu(x @ W1_e + b1_e) @ W2_e + b2_e
  with W1_e [512,512], W2_e [512,128]. Output per bank: [E,B,OUT] viewed
  as [B, E, 1, OUT].

Strategy: data-parallel over B across 8 NeuronCores (2048 rows/core),
expert weights replicated. On-chip layout keeps the contraction dim on
SBUF partitions:
  - x is fed pre-transposed (xT: [H on partitions, B free], bf16)
  - GEMM1 computes hT = W1^T-free-dim matmuls -> psum [h' chunk, b]
  - ACT applies relu + per-partition bias b1, casts to bf16
  - GEMM2 computes oT [OUT on partitions, b] with W2 as the stationary
    operand, DVE adds per-partition bias b2 in the psum->sbuf copy.
Host transposes oT -> o and reshapes to the reference layout.
bf16 inputs keep the PE at 1 cycle/row (fp32 would be 4x slower); fp32
accumulation in PSUM bounds the error at ~3e-3 scale-relative absmax.
"""

import numpy as np
import ml_dtypes

B = 16384
H = 512
E = 5
T = 2
OUT = 128
NB = 3                 # input banks: share, task0, task1
NE = NB * E            # 15 expert instances
NCORES = 8
BSH = B // NCORES      # 2048 rows per core
P = 128
KC = H // P            # 4 contraction chunks
MC = H // P            # 4 h' chunks
NT = BSH // 512        # 4 b tiles of 512

BF16 = ml_dtypes.bfloat16

_compiled = None       # cached (nc, ) across calls


def _build_program(repeat=None):
    """Build the per-core program. repeat=None emits the plain kernel;
    repeat=R wraps the body in a hardware For_i loop (timing rig only).
    """
    import concourse.mybir as mybir
    from concourse import bacc
    from concourse.tile import TileContext
    from contextlib import nullcontext

    f32 = mybir.dt.float32
    bf16 = mybir.dt.bfloat16

    nc = bacc.Bacc("TRN2", target_bir_lowering=False, debug=False,
                   num_devices=NCORES)

    xt_d = nc.declare_dram_parameter("xt", [NB, P, KC, BSH], bf16, isOutput=False)
    w1_d = nc.declare_dram_parameter("w1", [NE, P, KC, H], bf16, isOutput=False)
    w2_d = nc.declare_dram_parameter("w2", [NE, P, KC, OUT], bf16, isOutput=False)
    b1_d = nc.declare_dram_parameter("b1", [P, NE, MC], f32, isOutput=False)
    b2_d = nc.declare_dram_parameter("b2", [P, NE], f32, isOutput=False)
    out_d = nc.declare_dram_parameter("out", [NE, P, BSH], f32, isOutput=True)

    with TileContext(nc) as tc:
        loop_ctx = (tc.For_i(0, repeat, 1, hint_engines=(mybir.EngineType.PE,))
                    if repeat is not None else nullcontext())
        with (
            loop_ctx,
            tc.tile_pool(name="xpool", bufs=1) as xpool,
            tc.tile_pool(name="consts", bufs=1) as consts,
            tc.tile_pool(name="w1pool", bufs=3) as w1pool,
            tc.tile_pool(name="w2pool", bufs=3) as w2pool,
            tc.tile_pool(name="hpool", bufs=2) as hpool,
            tc.tile_pool(name="opool", bufs=2) as opool,
            tc.tile_pool(name="ps1", bufs=5, space="PSUM") as ps1,
            tc.tile_pool(name="ps2", bufs=3, space="PSUM") as ps2,
        ):
            # xT lives in per-(bank, n-tile) tiles so the first matmul
            # group only waits on its own 512KB slice, not all 6MB.
            xt_sb = {}

            def load_x(bank, n):
                t = xpool.tile([P, KC, 512], bf16, tag=f"xt{bank}_{n}")
                nc.sync.dma_start(
                    out=t[:], in_=xt_d[bank][:, :, n * 512:(n + 1) * 512])
                xt_sb[(bank, n)] = t

            # Critical-path DMAs first: first expert's W1, then bank0 x.
            w1_first = w1pool.tile([P, KC, H], bf16, tag="w1")
            nc.sync.dma_start(out=w1_first[:], in_=w1_d[0])
            load_x(0, 0)
            b1_sb = consts.tile([P, NE, MC], f32)
            nc.sync.dma_start(out=b1_sb[:], in_=b1_d[:])
            b2_sb = consts.tile([P, NE], f32)
            nc.sync.dma_start(out=b2_sb[:], in_=b2_d[:])
            for n in range(1, NT):
                load_x(0, n)

            def gemm1_group(i, bank, m, n, w1_sb, h_sb):
                ps = ps1.tile([P, 512], f32)
                for kc in range(KC):
                    nc.tensor.matmul(
                        ps[:],
                        w1_sb[:, kc, m * P:(m + 1) * P],
                        xt_sb[(bank, n)][:, kc, :],
                        start=(kc == 0),
                        stop=(kc == KC - 1),
                    )
                nc.scalar.activation(
                    h_sb[:, m, n * 512:(n + 1) * 512],
                    ps[:],
                    mybir.ActivationFunctionType.Relu,
                    bias=b1_sb[:, i, m, None],
                )

            def gemm2_group(j, n, w2_sb, h_sb, o_sb):
                ps = ps2.tile([P, 512], f32)
                for kc in range(KC):
                    nc.tensor.matmul(
                        ps[:],
                        w2_sb[:, kc, :],
                        h_sb[:, kc, n * 512:(n + 1) * 512],
                        start=(kc == 0),
                        stop=(kc == KC - 1),
                    )
                nc.vector.tensor_add(
                    o_sb[:, n * 512:(n + 1) * 512],
                    ps[:],
                    b2_sb[:, j, None].to_broadcast([P, 512]),
                )
                # Output DMAs ride the (idle) gpsimd SWDGE queue: they wait
                # on compute, and on the in-order SP queue that wait would
                # head-of-line-block the weight prefetches behind them.
                nc.gpsimd.dma_start(
                    out=out_d[j][:, n * 512:(n + 1) * 512],
                    in_=o_sb[:, n * 512:(n + 1) * 512],
                )

            live = {}  # step i -> (h tile, w2 tile)
            for step in range(NE + 1):
                if step < NE:
                    i = step
                    bank = i // E
                    if i == 0:
                        w1_sb = w1_first
                    else:
                        w1_sb = w1pool.tile([P, KC, H], bf16, tag="w1")
                        nc.sync.dma_start(out=w1_sb[:], in_=w1_d[i])
                    w2_sb = w2pool.tile([P, KC, OUT], bf16)
                    nc.sync.dma_start(out=w2_sb[:], in_=w2_d[i])
                    # Stream later banks' x while early experts compute.
                    if step == 1:
                        for n in range(NT):
                            load_x(1, n)
                    elif step == 5:
                        for n in range(NT):
                            load_x(2, n)
                    h_sb = hpool.tile([P, MC, BSH], bf16)
                    live[i] = (h_sb, w2_sb)
                    if i < NE - 1:
                        for m in range(MC):
                            for n in range(NT):
                                gemm1_group(i, bank, m, n, w1_sb, h_sb)
                    else:
                        # Last expert: n-outer so its GEMM2 groups can
                        # chase the relu wavefront and shorten the tail.
                        o_sb = opool.tile([P, BSH], f32)
                        live[i] = (h_sb, w2_sb, o_sb)
                        for n in range(NT):
                            for m in range(MC):
                                gemm1_group(i, bank, m, n, w1_sb, h_sb)
                            if n > 0:
                                gemm2_group(i, n - 1, w2_sb, h_sb, o_sb)
                if step > 0:
                    j = step - 1
                    if j < NE - 1:
                        h_sb, w2_sb = live.pop(j)
                        o_sb = opool.tile([P, BSH], f32)
                        for n in range(NT):
                            gemm2_group(j, n, w2_sb, h_sb, o_sb)
                    else:
                        h_sb, w2_sb, o_sb = live.pop(j)
                        gemm2_group(j, NT - 1, w2_sb, h_sb, o_sb)

    nc.compile()
    return nc


def _prep_inputs(share_x, task_x0, task_x1, share_W1, share_b1, share_W2,
                 share_b2, task_W1, task_b1, task_W2, task_b2):
    X = np.stack([np.asarray(share_x), np.asarray(task_x0),
                  np.asarray(task_x1)]).astype(np.float32)      # [3, B, H]
    Xb = X.astype(BF16)
    Xt = np.ascontiguousarray(Xb.transpose(0, 2, 1))            # [3, H, B]
    Xt = Xt.reshape(NB, KC, P, B).transpose(0, 2, 1, 3)         # [3, P, KC, B]

    W1 = np.concatenate([np.asarray(share_W1),
                         np.asarray(task_W1).reshape(T * E, H, H)])  # [15,H,H]
    w1h = np.ascontiguousarray(
        W1.astype(BF16).reshape(NE, KC, P, H).transpose(0, 2, 1, 3))
    W2 = np.concatenate([np.asarray(share_W2),
                         np.asarray(task_W2).reshape(T * E, H, OUT)])
    w2h = np.ascontiguousarray(
        W2.astype(BF16).reshape(NE, KC, P, OUT).transpose(0, 2, 1, 3))

    B1 = np.concatenate([np.asarray(share_b1),
                         np.asarray(task_b1).reshape(T * E, H)]).astype(np.float32)
    b1h = np.ascontiguousarray(B1.reshape(NE, MC, P).transpose(2, 0, 1))
    B2 = np.concatenate([np.asarray(share_b2),
                         np.asarray(task_b2).reshape(T * E, OUT)]).astype(np.float32)
    b2h = np.ascontiguousarray(B2.T)

    in_maps = []
    for c in range(NCORES):
        xt_c = np.ascontiguousarray(Xt[:, :, :, c * BSH:(c + 1) * BSH])
        in_maps.append({"xt": xt_c, "w1": w1h, "w2": w2h, "b1": b1h, "b2": b2h})
    return in_maps


def _assemble(results):
    outs = np.stack([results[c]["out"] for c in range(NCORES)])  # [8,15,P,BSH]
    # outs[c, i, p, b] = o_i[c*BSH + b, p] -> A[i, B, OUT]
    A = np.ascontiguousarray(outs.transpose(1, 0, 3, 2)).reshape(NE, B, OUT)
    banks = []
    for bank in range(NB):
        o = A[bank * E:(bank + 1) * E]                    # [E, B, OUT]
        banks.append(o.reshape(-1, E, 1, OUT))            # [B, E, 1, OUT]
    return tuple(banks)


def kernel(**inputs):
    global _compiled
    from concourse.bass_utils import run_bass_kernel_spmd

    if _compiled is None:
        _compiled = _build_program()
    nc = _compiled

    in_maps = _prep_inputs(**inputs)
    res = run_bass_kernel_spmd(nc, in_maps, list(range(NCORES)))
    return _assemble(res.results)
